# revision 5
# baseline (speedup 1.0000x reference)
"""AiLUT kernel for Trainium2 (8 NeuronCores, data-parallel) — SWDGE gather design.

Host computes the tiny backbone (resize->convs->IN->heads) in numpy (negligible
FLOPs), producing per-batch LUTs + vertices, then builds:
  - E [32768, 128] bf16 rows (256 B, the SWDGE dma_gather minimum element):
      f32 cols (bitcast) k=dg*6+db*3+c : corner L[c, ib+db, ig+dg, ir]
      f32 cols 12+c : v0_c   15+c : 1/max(v1_c-v0_c, 1e-10)
      bf16 cols 36+k : r-delta L[c,...,ir+1] - L[c,...,ir]
    for cell = (ib*32 + ig)*32 + ir.
  - vth [128, 192] f32: cols 32c+j = v_{j+1};  cols 96+32c+j = -v_{j+1}.

Device (Bass, SPMD on 8 cores; each core owns one (batch, row-block) quarter
= 1,048,576 pixels as [128, 8192]):
  - searchsorted: ch R = fused is_ge+add chain on DVE (f32); ch G/B = Sign ops
    on the ACT engine + bf16 tree-sums on DVE (sign-sum S = 2*count - 32).
  - cell = 512*SB + 16*SG + accR + 16896 -> int16.
  - fold to the SWDGE index layout (idx[q, 8t+u] = cell[16u+q, t]) via 8
    concat DMAs + one strided DVE copy, then replicated to all 8 16-partition
    stripes (the gather ucode reads the stripe of its vNC).
  - per-pixel 256 B row gather via gpsimd.dma_gather (single_packet, 1024
    idxs/call = the 16 KB/engine packet cap), rotated over SWDGE queues.
  - trilinear: pre-differenced r-lerp (M = C0 + fr*DD), then packed in-place
    g- and b-lerps in bf16; f32 output.
"""

import numpy as np
import ml_dtypes

V = 33
EPS = 1e-5

NIDX = 1024        # idxs per dma_gather (single_packet cap: 16 KB/engine)
TC = 512           # strip columns
TG = 128           # trilinear tile columns
NQ = 4             # SWDGE queues

# ----------------------------------------------------------------- host math


def _resize_bilinear_np(x):
    # 2048 -> 256, align_corners=False: src = (i+0.5)*8-0.5 -> i0 = 8i+3, f=0.5
    b, c, H, W = x.shape
    y0 = np.arange(256) * 8 + 3
    rows = x[:, :, y0, :] * 0.5 + x[:, :, y0 + 1, :] * 0.5
    return rows[:, :, :, y0] * 0.5 + rows[:, :, :, y0 + 1] * 0.5


def _conv_s2_np(x, w, bias):
    b, ci, H, W = x.shape
    co = w.shape[0]
    oh, ow = H // 2, W // 2
    xp = np.zeros((b, ci, H + 2, W + 2), np.float32)
    xp[:, :, 1 : H + 1, 1 : W + 1] = x
    y = np.zeros((b, co, oh, ow), np.float32)
    for ky in range(3):
        for kx in range(3):
            xs = xp[:, :, ky : ky + 2 * oh : 2, kx : kx + 2 * ow : 2]
            y += np.einsum("oi,biyx->boyx", w[:, :, ky, kx], xs,
                           dtype=np.float32, casting="same_kind")
    return y + bias[None, :, None, None]


def _lrelu_np(x):
    return np.where(x >= 0, x, np.float32(0.2) * x).astype(np.float32)


def _inorm_np(x, g, be):
    m = x.mean((2, 3), keepdims=True, dtype=np.float32)
    v = x.var((2, 3), keepdims=True, dtype=np.float32)
    return ((x - m) / np.sqrt(v + np.float32(EPS)) * g[None, :, None, None]
            + be[None, :, None, None]).astype(np.float32)


def _backbone_np(imgs, w1, b1, g1, be1, w2, b2, g2, be2, w3, b3, g3, be3,
                 w4, b4, g4, be4, w5, b5, wgen_w, wgen_b, basis_w, ada_w, ada_b):
    b = imgs.shape[0]
    x = _resize_bilinear_np(imgs).astype(np.float32)
    for (w, bb, g, be) in ((w1, b1, g1, be1), (w2, b2, g2, be2),
                           (w3, b3, g3, be3), (w4, b4, g4, be4)):
        x = _inorm_np(_lrelu_np(_conv_s2_np(x, w, bb)), g, be)
    x = _lrelu_np(_conv_s2_np(x, w5, b5))                     # (b,128,8,8)
    x = x.reshape(b, 128, 2, 4, 2, 4).mean((3, 5), dtype=np.float32)
    x = x.reshape(b, 512).astype(np.float32)
    weights = x @ wgen_w + wgen_b                             # (b,3)
    luts = (weights @ basis_w).reshape(b, 3, V, V, V).astype(np.float32)
    logits = (x @ ada_w + ada_b).reshape(b, 3, V - 1).astype(np.float32)
    e = np.exp(logits - logits.max(-1, keepdims=True))
    intervals = (e / e.sum(-1, keepdims=True)).astype(np.float32)
    vertices = np.concatenate(
        [np.zeros((b, 3, 1), np.float32), np.cumsum(intervals, -1)], -1
    ).astype(np.float32)                                      # (b,3,V)
    return luts, vertices


def _build_tables(luts, vertices):
    """Per batch: E [32768, 128] bf16 (f32 frac fields bitcast inside) and
    vth [128, 192] f32 (+v thresholds for DVE, -v biases for ACT Sign)."""
    b = luts.shape[0]
    ib, ig, ir = np.meshgrid(np.arange(32), np.arange(32), np.arange(32),
                             indexing="ij")
    etabs, vths = [], []
    for bi in range(b):
        L = luts[bi]                                  # (3, 33, 33, 33)
        Eb = np.zeros((32768, 128), ml_dtypes.bfloat16)
        Ef = Eb.view(np.float32)                      # (32768, 64)
        for dg in (0, 1):
            for db in (0, 1):
                blk = L[:, db:db + 32, dg:dg + 32, :]  # (3,32,32,33)
                base = blk[:, :, :, 0:32]
                dd = blk[:, :, :, 1:33] - base
                for c in range(3):
                    k = dg * 6 + db * 3 + c
                    Ef[:, k] = base[c].reshape(-1)          # c0: f32 cols 0..11
                    Eb[:, 36 + k] = dd[c].reshape(-1)       # dd: bf16 cols 36..47
        vtx = vertices[bi]                            # (3, 33)
        for c, ax in ((0, ir), (1, ig), (2, ib)):
            v0 = vtx[c][ax].reshape(-1)
            v1 = vtx[c][ax.reshape(-1) + 1]
            Ef[:, 12 + c] = v0
            Ef[:, 15 + c] = 1.0 / np.maximum(v1 - v0, 1e-10)
        etabs.append(Eb)
        vth = np.zeros((128, 192), np.float32)
        for c in range(3):
            vth[:, 32 * c : 32 * c + 32] = vtx[c][1:33][None, :]
            vth[:, 96 + 32 * c : 96 + 32 * c + 32] = -vtx[c][1:33][None, :]
        vths.append(vth)
    return etabs, vths


# ------------------------------------------------------------- device kernel

_CACHED_NC = None


def _build_device_program():
    global _CACHED_NC
    if _CACHED_NC is not None:
        return _CACHED_NC
    import concourse.bacc as bacc
    import concourse.mybir as mybir
    import concourse.tile as tile
    from concourse import library_config
    from concourse.mybir import AluOpType as op

    f32 = mybir.dt.float32
    bf16 = mybir.dt.bfloat16
    i16 = mybir.dt.int16
    AF = mybir.ActivationFunctionType

    W = 8192
    NS = W // TC                  # strips
    TPS = TC // TG                # trilinear tiles per strip
    GPT = (128 * TG) // NIDX      # gathers per tile
    IC = NIDX // 16               # idx columns per gather

    nc = bacc.Bacc("TRN2", target_bir_lowering=False, debug=False,
                   num_devices=8, num_swdge_queues=NQ)
    img = nc.dram_tensor("img", [3, 128, W], f32, kind="ExternalInput").ap()
    etab = nc.dram_tensor("etab", [32768, 128], bf16, kind="ExternalInput").ap()
    vth = nc.dram_tensor("vth", [128, 192], f32, kind="ExternalInput").ap()
    out = nc.dram_tensor("out", [3, 128, W], f32, kind="ExternalOutput").ap()

    qstate = [0]

    def next_q():
        q = qstate[0]
        qstate[0] = (q + 1) % NQ
        return q

    with tile.TileContext(nc) as tc:
        with tc.tile_pool(name="cst", bufs=1) as cst, \
             tc.tile_pool(name="xp", bufs=2) as xp, \
             tc.tile_pool(name="ss", bufs=2) as ssp, \
             tc.tile_pool(name="ix", bufs=2) as ixp, \
             tc.tile_pool(name="cc", bufs=1) as ccp, \
             tc.tile_pool(name="gg", bufs=2) as ggp, \
             tc.tile_pool(name="tt", bufs=2) as ttp:
            nc.gpsimd.load_library(library_config.mlp)
            vt = cst.tile([128, 192], f32)
            nc.sync.dma_start(vt[:], vth[:])

            for s in range(NS):
                s0 = s * TC
                X = xp.tile([128, 3, TC], f32, tag="X")
                for c in range(3):
                    nc.sync.dma_start(X[:, c, :], img[c, :, s0:s0 + TC])

                # channel 0 (R): fused is_ge+add chain, f32 on DVE
                aR0 = ssp.tile([128, TC], f32, tag="aR0")
                aR1 = ssp.tile([128, TC], f32, tag="aR1")
                nc.vector.tensor_scalar(out=aR0[:], in0=X[:, 0, :],
                                        scalar1=vt[:, 0:1], scalar2=0.0,
                                        op0=op.is_ge, op1=op.add)
                cur, nxt = aR0, aR1
                for j in range(1, 32):
                    nc.vector.scalar_tensor_tensor(
                        out=nxt[:], in0=X[:, 0, :], scalar=vt[:, j:j + 1],
                        in1=cur[:], op0=op.is_ge, op1=op.add)
                    cur, nxt = nxt, cur
                accR = cur
                nc.vector.tensor_scalar(out=accR[:], in0=accR[:], scalar1=31.0,
                                        scalar2=0.0, op0=op.min, op1=op.add)

                # channels 1 (G), 2 (B): ACT Sign batches + bf16 DVE trees
                accs = {}
                for c in (1, 2):
                    accS = ssp.tile([128, TC], bf16, tag=f"accS{c}")
                    for bch in range(4):
                        S8 = ssp.tile([128, 8, TC], bf16, tag="S8")
                        for j8 in range(8):
                            j = bch * 8 + j8
                            nc.scalar.activation(
                                out=S8[:, j8, :], in_=X[:, c, :], func=AF.Sign,
                                bias=vt[:, 96 + 32 * c + j:96 + 32 * c + j + 1],
                                scale=1.0)
                        P4 = ssp.tile([128, 4, TC], bf16, tag="P4")
                        nc.vector.tensor_tensor(out=P4[:], in0=S8[:, 0:4, :],
                                                in1=S8[:, 4:8, :], op=op.add)
                        P2 = ssp.tile([128, 2, TC], bf16, tag="P2")
                        nc.vector.tensor_tensor(out=P2[:], in0=P4[:, 0:2, :],
                                                in1=P4[:, 2:4, :], op=op.add)
                        if bch == 0:
                            nc.vector.tensor_tensor(out=accS[:], in0=P2[:, 0, :],
                                                    in1=P2[:, 1, :], op=op.add)
                        else:
                            P1 = ssp.tile([128, TC], bf16, tag="P1")
                            nc.vector.tensor_tensor(out=P1[:], in0=P2[:, 0, :],
                                                    in1=P2[:, 1, :], op=op.add)
                            nc.vector.tensor_tensor(out=accS[:], in0=accS[:],
                                                    in1=P1[:], op=op.add)
                    # count = (S+32)/2; route through i16 so a Sign(0) tie
                    # (odd S -> half-count) rounds to a whole adjacent index
                    accI = ssp.tile([128, TC], i16, tag=f"accI{c}")
                    nc.vector.tensor_scalar(out=accI[:], in0=accS[:],
                                            scalar1=32.0, scalar2=0.5,
                                            op0=op.add, op1=op.mult)
                    accF = ssp.tile([128, TC], f32, tag=f"accF{c}")
                    nc.vector.tensor_copy(accF[:], accI[:])
                    nc.vector.tensor_scalar(out=accF[:], in0=accF[:],
                                            scalar1=31.0, scalar2=0.0,
                                            op0=op.min, op1=op.add)
                    accs[c] = accF

                # cell = (accB*32 + accG)*32 + accR  -> int16
                t1 = ssp.tile([128, TC], f32, tag="t1")
                nc.vector.scalar_tensor_tensor(out=t1[:], in0=accs[2][:],
                                               scalar=32.0, in1=accs[1][:],
                                               op0=op.mult, op1=op.add)
                t2 = ssp.tile([128, TC], f32, tag="t2")
                nc.vector.scalar_tensor_tensor(out=t2[:], in0=t1[:],
                                               scalar=32.0, in1=accR[:],
                                               op0=op.mult, op1=op.add)
                celli = ssp.tile([128, TC], i16, tag="celli")
                nc.vector.tensor_copy(celli[:], t2[:])

                # fold to idx[q, 8t+u] = cell[16u+q, t], replicate to 8 stripes
                cat = ccp.tile([16, 8 * TC], i16, tag="cat")
                for u in range(8):
                    nc.sync.dma_start(cat[:, u * TC:(u + 1) * TC],
                                      celli[16 * u:16 * (u + 1), :])
                idx = ixp.tile([128, 8 * TC], i16, tag="idx")
                cat_v = cat[:].rearrange("p (u t) -> p t u", u=8)
                idx_v = idx[0:16, :].rearrange("p (t u) -> p t u", u=8)
                nc.vector.tensor_copy(idx_v, cat_v)
                nc.sync.dma_start(idx[16:32, :], idx[0:16, :])
                nc.sync.dma_start(idx[32:64, :], idx[0:32, :])
                nc.sync.dma_start(idx[64:128, :], idx[0:64, :])

                for k in range(TPS):
                    tg0 = k * TG
                    g = ggp.tile([128, TG, 128], bf16, tag="g")
                    gc = TG // GPT          # gathered columns per call
                    for h in range(GPT):
                        nc.gpsimd.dma_gather(
                            g[:, h * gc:(h + 1) * gc, :], etab[:],
                            idx[:, (k * GPT + h) * IC:(k * GPT + h + 1) * IC],
                            NIDX, NIDX, 128,
                            queue_num=next_q(), single_packet=True)
                    gf = g[:].bitcast(f32)                       # [128, TG, 64]
                    v0v = gf[:, :, 12:15].transpose([0, 2, 1])   # [128, 3, TG]
                    invv = gf[:, :, 15:18].transpose([0, 2, 1])
                    d3 = ttp.tile([128, 3, TG], f32, tag="d3")
                    nc.vector.tensor_tensor(out=d3[:], in0=X[:, :, tg0:tg0 + TG],
                                            in1=v0v, op=op.subtract)
                    nc.vector.tensor_tensor(out=d3[:], in0=d3[:], in1=invv,
                                            op=op.mult)
                    u3 = ttp.tile([128, 3, TG], f32, tag="u3")
                    nc.scalar.activation(out=u3[:], in_=d3[:], func=AF.Relu,
                                         bias=1.0, scale=-1.0)
                    f3 = ttp.tile([128, 3, TG], bf16, tag="f3")
                    nc.scalar.activation(out=f3[:], in_=u3[:], func=AF.Relu,
                                         bias=1.0, scale=-1.0)
                    # lvl1 (r): M = C0(f32) + fr*DD(bf16)  [128, 12, TG] f32
                    M = ttp.tile([128, 12, TG], f32, tag="M")
                    ddv = g[:, :, 36:48].transpose([0, 2, 1])
                    c0v = gf[:, :, 0:12].transpose([0, 2, 1])
                    frb = f3[:, 0:1, :].broadcast_to((128, 12, TG))
                    nc.vector.tensor_tensor(out=M[:], in0=ddv, in1=frb,
                                            op=op.mult)
                    nc.vector.tensor_tensor(out=M[:], in0=M[:], in1=c0v,
                                            op=op.add)
                    # lvl2 (g), in place on M
                    fgb = f3[:, 1:2, :].broadcast_to((128, 6, TG))
                    nc.vector.tensor_tensor(out=M[:, 6:12, :], in0=M[:, 6:12, :],
                                            in1=M[:, 0:6, :], op=op.subtract)
                    nc.vector.tensor_tensor(out=M[:, 6:12, :], in0=M[:, 6:12, :],
                                            in1=fgb, op=op.mult)
                    nc.vector.tensor_tensor(out=M[:, 0:6, :], in0=M[:, 0:6, :],
                                            in1=M[:, 6:12, :], op=op.add)
                    # lvl3 (b)
                    fbb = f3[:, 2:3, :].broadcast_to((128, 3, TG))
                    nc.vector.tensor_tensor(out=M[:, 3:6, :], in0=M[:, 3:6, :],
                                            in1=M[:, 0:3, :], op=op.subtract)
                    nc.vector.tensor_tensor(out=M[:, 3:6, :], in0=M[:, 3:6, :],
                                            in1=fbb, op=op.mult)
                    O = ttp.tile([128, 3, TG], f32, tag="O")
                    nc.vector.tensor_tensor(out=O[:], in0=M[:, 0:3, :],
                                            in1=M[:, 3:6, :], op=op.add)
                    for c in range(3):
                        nc.sync.dma_start(out[c, :, s0 + tg0:s0 + tg0 + TG],
                                          O[:, c, :])

    nc.compile()
    _CACHED_NC = nc
    return nc


# ------------------------------------------------------------------- kernel


def kernel(**inputs):
    imgs = np.asarray(inputs["imgs"], np.float32)
    assert imgs.shape == (2, 3, 2048, 2048)

    luts, vertices = _backbone_np(
        imgs,
        *[np.asarray(inputs[k], np.float32) for k in
          ("w1", "b1", "g1", "be1", "w2", "b2", "g2", "be2",
           "w3", "b3", "g3", "be3", "w4", "b4", "g4", "be4", "w5", "b5",
           "wgen_w", "wgen_b", "basis_w", "ada_w", "ada_b")])
    etabs, vths = _build_tables(luts, vertices)

    nc = _build_device_program()
    from concourse.bass_utils import run_bass_kernel_spmd

    in_maps = []
    for core in range(8):
        bi, blk = core // 4, core % 4
        sl = imgs[bi, :, 512 * blk : 512 * (blk + 1), :]       # [3,512,2048]
        in_maps.append({
            "img": np.ascontiguousarray(sl).reshape(3, 128, 8192),
            "etab": etabs[bi],
            "vth": vths[bi],
        })
    res = run_bass_kernel_spmd(nc, in_maps, core_ids=list(range(8)))

    outp = np.zeros((2, 3, 2048, 2048), np.float32)
    for core in range(8):
        bi, blk = core // 4, core % 4
        outp[bi, :, 512 * blk : 512 * (blk + 1), :] = (
            np.asarray(res.results[core]["out"]).reshape(3, 512, 2048))
    return outp
